# revision 3
# baseline (speedup 1.0000x reference)
"""Trainium2 Bass kernel for ternary-weight linear (plinear STE forward).

Reference math:
    y = x @ ((w_pos > 0) - (w_neg > 0)).T      # [8192, 4096]

Algebraic fold: the two binarized matmuls collapse into ONE matmul with a
ternary {-1,0,1} weight matrix, halving PE work. Ternary values are exact in
bf16, so the matmul runs at bf16 rate (2x fp32); only x is quantized to bf16.

Sharding (8 cores): 2 token-shards x 4 out-feature shards.
Per core: x_shard [4096, 4096] (bf16, staged transposed so K=in_features lands
on SBUF partitions), w slices [4096, 1024] (bf16, staged transposed),
binarize+subtract on device (DVE), then a K=4096 accumulated matmul with
x tiles stationary and ternary weights moving. Output [4096, 1024] fp32.
"""

import numpy as np
import ml_dtypes

P = 128
N_TOK, IN_F, OUT_F = 8192, 4096, 4096
TA, OB = 2, 4                 # token shards x out shards = 8 cores
T_S = N_TOK // TA             # 4096 tokens per shard
O_S = OUT_F // OB             # 1024 out features per shard
K_SUB = IN_F // P             # 32 k-subtiles
T_TILE = 256                  # tokens per streamed x tile
N_TT = T_S // T_TILE          # 16
N_FREE = 512                  # matmul moving free dim (one PSUM bank of fp32)

_CACHE = {}


def _build(repeats=1):
    key = ("nc", repeats)
    if key in _CACHE:
        return _CACHE[key]
    import concourse.bacc as bacc
    import concourse.mybir as mybir
    import concourse.tile as tile
    from concourse.bass import ds

    nc = bacc.Bacc("TRN2", target_bir_lowering=False, debug=False)
    xT = nc.dram_tensor("xT", (IN_F, T_S), mybir.dt.bfloat16, kind="ExternalInput")
    wpT = nc.dram_tensor("wpT", (IN_F, O_S), mybir.dt.bfloat16, kind="ExternalInput")
    wnT = nc.dram_tensor("wnT", (IN_F, O_S), mybir.dt.bfloat16, kind="ExternalInput")
    y = nc.dram_tensor("y", (T_S, O_S), mybir.dt.float32, kind="ExternalOutput")

    xT_r = xT[:].rearrange("(ko ki) t -> ki ko t", ki=P)     # [128, 32, 4096]
    wpT_r = wpT[:].rearrange("(ko ki) o -> ki ko o", ki=P)   # [128, 32, 1024]
    wnT_r = wnT[:].rearrange("(ko ki) o -> ki ko o", ki=P)
    y_r = y[:].rearrange("(to ti) o -> ti to o", ti=P)       # [128, 32, 1024]

    with tile.TileContext(nc) as tc:
        with (
            tc.tile_pool(name="tern", bufs=1) as tern_pool,
            tc.tile_pool(name="wstage", bufs=3) as wstage,
            tc.tile_pool(name="xp", bufs=3) as xp,
            tc.tile_pool(name="outp", bufs=3) as outp,
            tc.tile_pool(name="psum", bufs=4, space="PSUM") as psum_pool,
        ):
            for _rep in range(repeats):
                # ---- Phase A: ternary weights, K-major, SBUF-resident ----
                ternT = tern_pool.tile([P, K_SUB, O_S], mybir.dt.bfloat16)
                for k in range(K_SUB):
                    wp_t = wstage.tile([P, O_S], mybir.dt.bfloat16, tag="w")
                    wn_t = wstage.tile([P, O_S], mybir.dt.bfloat16, tag="w")
                    nc.sync.dma_start(wp_t[:], wpT_r[:, k, :])
                    nc.sync.dma_start(wn_t[:], wnT_r[:, k, :])
                    bn = wstage.tile([P, O_S], mybir.dt.bfloat16, tag="b")
                    nc.vector.tensor_scalar(
                        bn[:], wn_t[:], 0.0, None, mybir.AluOpType.is_gt
                    )
                    # ternT = (wp > 0) - (wn > 0)
                    nc.vector.scalar_tensor_tensor(
                        ternT[:, k, :], wp_t[:], 0.0, bn[:],
                        mybir.AluOpType.is_gt, mybir.AluOpType.subtract,
                    )

                # ---- Phase B: y[t, o] = sum_k xT[k, t] * ternT[k, o] ----
                for tt in range(N_TT):
                    x_t = xp.tile([P, K_SUB, T_TILE], mybir.dt.bfloat16)
                    nc.sync.dma_start(x_t[:], xT_r[:, :, ds(tt * T_TILE, T_TILE)])
                    for m in range(T_TILE // P):
                        ps = psum_pool.tile([P, O_S], mybir.dt.float32)
                        for k in range(K_SUB):
                            for ob2 in range(O_S // N_FREE):
                                nc.tensor.matmul(
                                    ps[:, ob2 * N_FREE:(ob2 + 1) * N_FREE],
                                    x_t[:, k, m * P:(m + 1) * P],
                                    ternT[:, k, ob2 * N_FREE:(ob2 + 1) * N_FREE],
                                    start=(k == 0),
                                    stop=(k == K_SUB - 1),
                                )
                        o_t = outp.tile([P, O_S], mybir.dt.float32)
                        nc.vector.tensor_copy(o_t[:], ps[:])
                        nc.sync.dma_start(
                            y_r[:, tt * (T_TILE // P) + m, :], o_t[:])
    nc.compile()
    _CACHE[key] = nc
    return nc


def _shard_inputs(x, w_pos, w_neg):
    bf16 = ml_dtypes.bfloat16
    xT = np.ascontiguousarray(x.astype(bf16).T)       # [IN_F, N_TOK]
    wpT = np.ascontiguousarray(w_pos.astype(bf16).T)  # [IN_F, OUT_F]
    wnT = np.ascontiguousarray(w_neg.astype(bf16).T)
    in_maps = []
    for c in range(TA * OB):
        ta, ob = divmod(c, OB)
        in_maps.append({
            "xT": np.ascontiguousarray(xT[:, ta * T_S:(ta + 1) * T_S]),
            "wpT": np.ascontiguousarray(wpT[:, ob * O_S:(ob + 1) * O_S]),
            "wnT": np.ascontiguousarray(wnT[:, ob * O_S:(ob + 1) * O_S]),
        })
    return in_maps


def run(x, w_pos, w_neg, trace=False):
    """Returns (y_full, BassKernelResults)."""
    from concourse import bass_utils

    nc = _build()
    in_maps = _shard_inputs(x, w_pos, w_neg)
    res = bass_utils.run_bass_kernel_spmd(
        nc, in_maps, core_ids=list(range(TA * OB)), trace=trace
    )
    y_full = np.empty((N_TOK, OUT_F), np.float32)
    for c in range(TA * OB):
        ta, ob = divmod(c, OB)
        y_full[ta * T_S:(ta + 1) * T_S, ob * O_S:(ob + 1) * O_S] = res.results[c]["y"]
    return y_full, res


def kernel(x, w_pos, w_neg):
    y, _ = run(x, w_pos, w_neg, trace=False)
    return y


# revision 6
# speedup vs baseline: 22.2958x; 22.2958x over previous
"""Trainium2 Bass kernel for ternary-weight linear (plinear STE forward).

Reference math:
    y = x @ ((w_pos > 0) - (w_neg > 0)).T      # [8192, 4096]

Algebraic fold: the two binarized matmuls collapse into ONE matmul with a
ternary {-1,0,1} weight matrix, halving PE work. Ternary values are exact in
bf16, so the matmul runs at bf16 rate (2x fp32); only x is quantized to bf16.

Sharding (8 cores): 2 token-shards x 4 out-feature shards.
Per core: x_shard [4096, 4096] (bf16, staged transposed so K=in_features lands
on SBUF partitions), w slices [4096, 1024] (bf16, staged transposed),
binarize+subtract on device (DVE), then a K=4096 accumulated matmul with
x tiles stationary and ternary weights moving. Output [4096, 1024] fp32.
"""

import numpy as np
import ml_dtypes

P = 128
N_TOK, IN_F, OUT_F = 8192, 4096, 4096
TA, OB = 2, 4                 # token shards x out shards = 8 cores
T_S = N_TOK // TA             # 4096 tokens per shard
O_S = OUT_F // OB             # 1024 out features per shard
K_SUB = IN_F // P             # 32 k-subtiles
T_TILE = 256                  # tokens per streamed x tile
N_TT = T_S // T_TILE          # 16
N_FREE = 512                  # matmul moving free dim (one PSUM bank of fp32)

_CACHE = {}


def _build(repeats=1):
    key = ("nc", repeats)
    if key in _CACHE:
        return _CACHE[key]
    import concourse.bacc as bacc
    import concourse.mybir as mybir
    import concourse.tile as tile
    from concourse.bass import ds

    nc = bacc.Bacc("TRN2", target_bir_lowering=False, debug=False)
    # x pre-tiled on host: [tt, ki, ko, t] so each tile DMA is one
    # contiguous 16KB line per partition (no 512B scatter).
    xP = nc.dram_tensor("xP", (N_TT, P, K_SUB, T_TILE), mybir.dt.bfloat16,
                        kind="ExternalInput")
    wpT = nc.dram_tensor("wpT", (IN_F, O_S), mybir.dt.bfloat16, kind="ExternalInput")
    wnT = nc.dram_tensor("wnT", (IN_F, O_S), mybir.dt.bfloat16, kind="ExternalInput")
    y = nc.dram_tensor("y", (T_S, O_S), mybir.dt.float32, kind="ExternalOutput")

    wpT_r = wpT[:].rearrange("(ko ki) o -> ki ko o", ki=P)   # [128, 32, 1024]
    wnT_r = wnT[:].rearrange("(ko ki) o -> ki ko o", ki=P)
    y_r = y[:].rearrange("(to ti) o -> ti to o", ti=P)       # [128, 32, 1024]

    with tile.TileContext(nc) as tc:
        with (
            tc.tile_pool(name="tern", bufs=1) as tern_pool,
            tc.tile_pool(name="wstage", bufs=3) as wstage,
            tc.tile_pool(name="xp", bufs=3) as xp,
            tc.tile_pool(name="outp", bufs=3) as outp,
            tc.tile_pool(name="psum", bufs=4, space="PSUM") as psum_pool,
        ):
            for _rep in range(repeats):
                # ---- Phase A: ternary weights, K-major, SBUF-resident ----
                ternT = tern_pool.tile([P, K_SUB, O_S], mybir.dt.bfloat16)
                for k in range(K_SUB):
                    wp_t = wstage.tile([P, O_S], mybir.dt.bfloat16, tag="w")
                    wn_t = wstage.tile([P, O_S], mybir.dt.bfloat16, tag="w")
                    nc.sync.dma_start(wp_t[:], wpT_r[:, k, :])
                    nc.sync.dma_start(wn_t[:], wnT_r[:, k, :])
                    bn = wstage.tile([P, O_S], mybir.dt.bfloat16, tag="b")
                    nc.vector.tensor_scalar(
                        bn[:], wn_t[:], 0.0, None, mybir.AluOpType.is_gt
                    )
                    # ternT = (wp > 0) - (wn > 0)
                    nc.vector.scalar_tensor_tensor(
                        ternT[:, k, :], wp_t[:], 0.0, bn[:],
                        mybir.AluOpType.is_gt, mybir.AluOpType.subtract,
                    )

                # ---- Phase B: y[t, o] = sum_k xT[k, t] * ternT[k, o] ----
                for tt in range(N_TT):
                    x_t = xp.tile([P, K_SUB, T_TILE], mybir.dt.bfloat16)
                    nc.sync.dma_start(x_t[:], xP[tt])
                    for m in range(T_TILE // P):
                        ps = psum_pool.tile([P, O_S], mybir.dt.float32)
                        for k in range(K_SUB):
                            for ob2 in range(O_S // N_FREE):
                                nc.tensor.matmul(
                                    ps[:, ob2 * N_FREE:(ob2 + 1) * N_FREE],
                                    x_t[:, k, m * P:(m + 1) * P],
                                    ternT[:, k, ob2 * N_FREE:(ob2 + 1) * N_FREE],
                                    start=(k == 0),
                                    stop=(k == K_SUB - 1),
                                )
                        o_t = outp.tile([P, O_S], mybir.dt.float32)
                        nc.vector.tensor_copy(o_t[:], ps[:])
                        nc.sync.dma_start(
                            y_r[:, tt * (T_TILE // P) + m, :], o_t[:])
    nc.compile()
    _CACHE[key] = nc
    return nc


def _shard_inputs(x, w_pos, w_neg):
    bf16 = ml_dtypes.bfloat16
    xb = x.astype(bf16)                               # [N_TOK, IN_F]
    wpT = np.ascontiguousarray(w_pos.astype(bf16).T)  # [IN_F, OUT_F]
    wnT = np.ascontiguousarray(w_neg.astype(bf16).T)
    in_maps = []
    for c in range(TA * OB):
        ta, ob = divmod(c, OB)
        xs = xb[ta * T_S:(ta + 1) * T_S]              # [T_S, IN_F]
        # [tt, t, ko, ki] -> [tt, ki, ko, t]
        xp = np.ascontiguousarray(
            xs.reshape(N_TT, T_TILE, K_SUB, P).transpose(0, 3, 2, 1))
        in_maps.append({
            "xP": xp,
            "wpT": np.ascontiguousarray(wpT[:, ob * O_S:(ob + 1) * O_S]),
            "wnT": np.ascontiguousarray(wnT[:, ob * O_S:(ob + 1) * O_S]),
        })
    return in_maps


def run(x, w_pos, w_neg, trace=False):
    """Returns (y_full, BassKernelResults)."""
    from concourse import bass_utils

    nc = _build()
    in_maps = _shard_inputs(x, w_pos, w_neg)
    res = bass_utils.run_bass_kernel_spmd(
        nc, in_maps, core_ids=list(range(TA * OB)), trace=trace
    )
    y_full = np.empty((N_TOK, OUT_F), np.float32)
    for c in range(TA * OB):
        ta, ob = divmod(c, OB)
        y_full[ta * T_S:(ta + 1) * T_S, ob * O_S:(ob + 1) * O_S] = res.results[c]["y"]
    return y_full, res


def kernel(x, w_pos, w_neg):
    y, _ = run(x, w_pos, w_neg, trace=False)
    return y
